# revision 1
# baseline (speedup 1.0000x reference)
"""Data-parallel Trainium kernel for nn_Attention_5394478924244.

Teacher-forced additive-attention LSTM decoder. Sharding: batch B=256 is
split across the 8 NeuronCores (32 rows each); all weights are replicated.
The 26-step decode scan runs independently per batch shard, so no
collectives are needed. Inputs arrive full-shape; output is gathered back
to the full [B, STEPS, V] array.
"""
import numpy as np
import jax
import jax.numpy as jnp
from functools import partial

B, T, D = 256, 256, 512
H, V = 512, 96
STEPS = 26
NCORES = 8
BS = B // NCORES  # 32 rows per core


def _shard_step(batch_H, text, W_i2h, W_h2h, b_h2h, w_score, W_ih, W_hh,
                b_ih, b_hh, W_gen, b_gen):
    # batch_H: [BS, T, D]  text: [BS, STEPS]
    proj_H = jnp.einsum('btd,hd->bth', batch_H, W_i2h)          # [BS, T, H]
    onehots = jax.nn.one_hot(text.T, V, dtype=batch_H.dtype)    # [STEPS, BS, V]

    def step(carry, oh):
        h, c = carry
        proj_h = h @ W_h2h.T + b_h2h                            # [BS, H]
        e = jnp.tanh(proj_H + proj_h[:, None, :]) @ w_score     # [BS, T]
        alpha = jax.nn.softmax(e, axis=1)
        context = jnp.einsum('bt,btd->bd', alpha, batch_H)      # [BS, D]
        x = jnp.concatenate([context, oh], axis=1)              # [BS, D+V]
        gates = x @ W_ih.T + b_ih + h @ W_hh.T + b_hh           # [BS, 4H]
        i, f, g, o = jnp.split(gates, 4, axis=1)
        c_new = jax.nn.sigmoid(f) * c + jax.nn.sigmoid(i) * jnp.tanh(g)
        h_new = jax.nn.sigmoid(o) * jnp.tanh(c_new)
        logits = h_new @ W_gen.T + b_gen                        # [BS, V]
        return (h_new, c_new), logits

    init = (jnp.zeros((BS, H), batch_H.dtype), jnp.zeros((BS, H), batch_H.dtype))
    _, logits = jax.lax.scan(step, init, onehots)               # [STEPS, BS, V]
    return jnp.transpose(logits, (1, 0, 2))                     # [BS, STEPS, V]


_pmapped = None


def _get_pmapped():
    global _pmapped
    if _pmapped is None:
        _pmapped = jax.pmap(
            _shard_step,
            in_axes=(0, 0, None, None, None, None, None, None, None, None,
                     None, None),
            devices=jax.devices()[:NCORES],
        )
    return _pmapped


def kernel(batch_H, W_i2h, W_h2h, b_h2h, w_score, W_ih, W_hh, b_ih, b_hh,
           W_gen, b_gen, text):
    batch_H = np.asarray(batch_H, dtype=np.float32).reshape(NCORES, BS, T, D)
    text_sh = np.asarray(text).reshape(NCORES, BS, STEPS)
    fn = _get_pmapped()
    out = fn(batch_H, text_sh,
             jnp.asarray(W_i2h, jnp.float32), jnp.asarray(W_h2h, jnp.float32),
             jnp.asarray(b_h2h, jnp.float32), jnp.asarray(w_score, jnp.float32),
             jnp.asarray(W_ih, jnp.float32), jnp.asarray(W_hh, jnp.float32),
             jnp.asarray(b_ih, jnp.float32), jnp.asarray(b_hh, jnp.float32),
             jnp.asarray(W_gen, jnp.float32), jnp.asarray(b_gen, jnp.float32))
    return np.asarray(out).reshape(B, STEPS, V).astype(np.float32)
